# revision 8
# baseline (speedup 1.0000x reference)
"""Trainium2 Bass kernel: bidirectional conv-BN-relu message passing over H.

Reference semantics (per batch item, channels C, scan over H):
  forward:  new[0] = x[0];   new[h] = relu(bn(conv(new[h-1]))) + x[h]
  backward: out[H-1] = new[H-1]; out[h] = relu(bn(conv(out[h+1]))) + new[h]
conv = 1D conv along W, kernel 9, pad 4, C->C channels. BN folded to
per-channel scale/shift.

Strategy: data-parallel over B across 8 cores (2 batch items per core).
Each conv step = 9 shifted-window f32r matmuls accumulated in PSUM
(lhsT = per-tap [I,O] weights, rhs = padded state slice windows), then
two DVE ops using relu(s*y+t) = s*relu(y + t/s) (s>0):
  u = max(psum + t/s, 0);  new = u*s + carry.
The whole `new` tensor [C, H, W+8] stays in SBUF per chain (padded
slices double as matmul inputs); the backward pass overwrites slices
in place and streams results to DRAM.
"""

import os
from contextlib import ExitStack

import numpy as np

import bass_rust
import concourse.bass as bass
import concourse.tile as tile
from concourse import mybir
from concourse.bass_utils import run_bass_kernel_spmd

B, C, H, W = 16, 128, 64, 256
K, PAD = 9, 4
NCORES = 8
BPC = B // NCORES  # batch items per core
WP = W + 2 * PAD
EPS = 1e-5

F32 = mybir.dt.float32
F32R = mybir.dt.float32r

_NC_CACHE: dict = {}
LAST_RESULTS = None  # stashed BassKernelResults for test.py introspection


def _build_nc(bpc=BPC, h_dim=H, w_dim=W, mm_dtype=F32R):
    # Tensors feeding f32r matmuls are declared float32r end-to-end (same
    # 4-byte layout as fp32 host-side); the BIR verifier requires every
    # producer of matmul operands to emit float32r.
    wp = w_dim + 2 * PAD
    nc = bass.Bass()
    x_d = nc.dram_tensor("x", [bpc, h_dim, C, w_dim], mm_dtype, kind="ExternalInput")
    w_d = nc.dram_tensor("w", [C, K, C], mm_dtype, kind="ExternalInput")
    s_d = nc.dram_tensor("s", [C, 1], F32, kind="ExternalInput")
    b_d = nc.dram_tensor("b", [C, 1], F32, kind="ExternalInput")
    o_d = nc.dram_tensor("o", [bpc, h_dim, C, w_dim], mm_dtype, kind="ExternalOutput")

    add = mybir.AluOpType.add
    mx = mybir.AluOpType.max
    mult = mybir.AluOpType.mult

    with ExitStack() as ctx:
        tc = ctx.enter_context(tile.TileContext(nc))
        singles = ctx.enter_context(tc.tile_pool(name="singles", bufs=1))
        big = ctx.enter_context(tc.tile_pool(name="big", bufs=1))
        xs_pool = ctx.enter_context(tc.tile_pool(name="xs", bufs=8))
        u_pool = ctx.enter_context(tc.tile_pool(name="u", bufs=4))
        pp = ctx.enter_context(tc.tile_pool(name="pp", bufs=4, space="PSUM"))

        wt = singles.tile([C, K, C], mm_dtype, tag="wt", name="wt")
        nc.sync.dma_start(out=wt, in_=w_d[:, :, :])
        st = singles.tile([C, 1], F32, tag="st", name="st")
        nc.sync.dma_start(out=st, in_=s_d[:, :])
        bt = singles.tile([C, 1], F32, tag="bt", name="bt")
        nc.sync.dma_start(out=bt, in_=b_d[:, :])

        wr = [wt[:, k, :] for k in range(K)]

        # memset can't write float32r; zero an fp32 tile and copy-cast the
        # pad columns from it instead.
        zp = singles.tile([C, h_dim, 2 * PAD], F32, tag="zp", name="zp")
        nc.vector.memset(zp, 0.0)

        new = []
        for c in range(bpc):
            nt = big.tile([C, h_dim, wp], mm_dtype, tag=f"new{c}", name=f"new{c}")
            nc.vector.tensor_copy(out=nt[:, :, 0:PAD], in_=zp[:, :, 0:PAD])
            nc.vector.tensor_copy(
                out=nt[:, :, PAD + w_dim : wp], in_=zp[:, :, PAD : 2 * PAD]
            )
            new.append(nt)
            nc.sync.dma_start(out=nt[:, 0, PAD : PAD + w_dim], in_=x_d[c, 0])

        def conv_step(c, src_slice, dst_slice, carry_ap):
            # dst = relu(scale*conv(src) + shift) + carry
            pt = pp.tile([C, w_dim], F32, tag="pt", name="pt")
            for k in range(K):
                nc.tensor.matmul(
                    pt,
                    wr[k],
                    src_slice[:, k : k + w_dim],
                    start=(k == 0),
                    stop=(k == K - 1),
                )
            u = u_pool.tile([C, w_dim], F32, tag="u", name="u")
            nc.vector.tensor_scalar(
                out=u, in0=pt, scalar1=bt, scalar2=0.0, op0=add, op1=mx
            )
            nc.vector.scalar_tensor_tensor(
                out=dst_slice, in0=u, scalar=st, in1=carry_ap, op0=mult, op1=add
            )

        # Forward scan over H (both chains interleaved per h).
        for h in range(1, h_dim):
            for c in range(bpc):
                xs = xs_pool.tile([C, w_dim], mm_dtype, tag="xs", name="xs")
                nc.sync.dma_start(out=xs, in_=x_d[c, h])
                conv_step(
                    c,
                    new[c][:, h - 1, :],
                    new[c][:, h, PAD : PAD + w_dim],
                    xs,
                )

        # Backward scan; out[h] overwrites new[h] in place, then streams out.
        for c in range(bpc):
            nc.scalar.dma_start(
                out=o_d[c, h_dim - 1], in_=new[c][:, h_dim - 1, PAD : PAD + w_dim]
            )
        for h in range(h_dim - 2, -1, -1):
            for c in range(bpc):
                conv_step(
                    c,
                    new[c][:, h + 1, :],
                    new[c][:, h, PAD : PAD + w_dim],
                    new[c][:, h, PAD : PAD + w_dim],
                )
                nc.scalar.dma_start(
                    out=o_d[c, h], in_=new[c][:, h, PAD : PAD + w_dim]
                )

    # TRN2 caps most instructions at one semaphore wait (matmuls lower to an
    # LDWEIGHTS struct with a single wait slot); split any excess onto
    # EventSemaphore instructions like bacc does.
    bass_rust.generate_event_semaphores(nc)
    return nc


def _get_nc():
    key = (BPC, H, W)
    if key not in _NC_CACHE:
        _NC_CACHE[key] = _build_nc()
    return _NC_CACHE[key]


def _prep_params(conv_w, gamma, beta, run_mean, run_var):
    scale = gamma.astype(np.float64) / np.sqrt(run_var.astype(np.float64) + EPS)
    shift = beta.astype(np.float64) - run_mean.astype(np.float64) * scale
    s32 = scale.astype(np.float32).reshape(C, 1)
    b32 = (shift / scale).astype(np.float32).reshape(C, 1)
    w_t = np.ascontiguousarray(conv_w.transpose(1, 2, 0)).astype(np.float32)
    return w_t, s32, b32


def kernel(inputs, conv_w, gamma, beta, run_mean, run_var):
    global LAST_RESULTS
    w_t, s32, b32 = _prep_params(conv_w, gamma, beta, run_mean, run_var)
    xt = np.ascontiguousarray(np.asarray(inputs).transpose(0, 2, 1, 3))  # [B,H,C,W]
    in_maps = [
        dict(
            x=np.ascontiguousarray(xt[c * BPC : (c + 1) * BPC]),
            w=w_t,
            s=s32,
            b=b32,
        )
        for c in range(NCORES)
    ]
    nc = _get_nc()
    trace = os.environ.get("KERNEL_TRACE", "0") == "1"
    res = run_bass_kernel_spmd(
        nc, in_maps, core_ids=list(range(NCORES)), trace=trace
    )
    LAST_RESULTS = res
    out = np.concatenate([res.results[c]["o"] for c in range(NCORES)], axis=0)
    return np.ascontiguousarray(out.transpose(0, 2, 1, 3))
